# revision 1
# baseline (speedup 1.0000x reference)
"""Trainium2 Bass kernel for CombinedLoss (mse + bone_mse + hole_mse).

loss = mean(diff^2) + mean((bone*diff)^2) + mean((hole_dil*diff)^2)
with diff = y_pred - y_true, binary masks, and hole_dil a 15^3 binary box
dilation of hole0 = (y_true>=0.5)&(x<0.5).

Strategy: data-parallel over the D axis across 8 NeuronCores with an
8-left / 8-right slice halo (host zero-padded; 8 not 7 so slice pairs stay
even-aligned -> ops batch two D-slices per instruction). All cores run an
identical SPMD program. Masks are binary so (m*diff)^2 == m*diff^2 and the
loss collapses to sum(diff^2 * (1 + bone + hole_dil)) / N; each core emits
per-partition partial sums, summed on the host.

Inputs are cast to bf16 on the host: comparisons against 0.5 and the
dilation stay exact (binary/integer math), only diff picks up unbiased
input rounding (~4e-4 on the loss), and DMA traffic halves (memory-bound).

Layout: SBUF tiles are [128 part, 2 (d in pair), 2 (j), 256 (w)] with
partition p holding H row-pair (2p, 2p+j) -- each partition's DMA row is
1 KiB contiguous DRAM. The H-band matmul matrices are permuted to match.

Dilation = separable box SUM with one threshold at the end of the D->H
chain (counts are exact small integers in bf16/f32):
  - D pass: running-window sum over slices (bf16 integers <= 15, exact)
  - H pass: TensorEngine matmul with banded ones matrices (PSUM f32)
  - threshold: ScalarE Sign (counts >= 0 -> {0,1})
  - W pass: binary max log-tree (shifts 1,2,4,7) on zero-padded tiles
"""

import os
import sys

import numpy as np

sys.path.insert(0, "/opt/trn_rl_repo")

D_FULL, H, W = 256, 256, 256
NCORES = 8
SLAB = D_FULL // NCORES          # 32 own slices per core
HALO = 7
LPAD = 8                         # left halo padding (8 keeps pairs aligned)
HSLAB = SLAB + 2 * LPAD          # 48 haloed slices; own slice d = index d+8
WPAD = W + 2 * HALO              # 270 padded W extent for the max tree
NTOT = float(D_FULL * H * W)

LAST_EXEC_NS = None
LAST_RESULT = None

_NC_CACHE = {}


def _band_blocks() -> np.ndarray:
    """lhsT blocks for the H-axis banded box-sum matmul, [128, 4*128] f32.

    Interleaved-H layout: partition p of a k/m block b holds H row 2p+b.
    Block (b_k, b_m) at [:, 128*(2*b_k+b_m):...]:
      B[k', m'] = 1 iff |(2k'+b_k) - (2m'+b_m)| <= 7.
    """
    k = np.arange(128)[:, None]
    m = np.arange(128)[None, :]
    blocks = []
    for b_k in (0, 1):
        for b_m in (0, 1):
            blocks.append((np.abs((2 * k + b_k) - (2 * m + b_m)) <= HALO))
    return np.concatenate(blocks, axis=1).astype(np.float32)


def _build_nc():
    import concourse.bacc as bacc
    import concourse.mybir as mybir
    from concourse.tile import TileContext

    fp32 = mybir.dt.float32
    bf16 = mybir.dt.bfloat16
    OP = mybir.AluOpType
    ACT = mybir.ActivationFunctionType

    # Bacc (not raw Bass): its finalize() runs generate_event_semaphores(),
    # which splits >1-wait instructions into EventSemaphore prefixes -- the
    # TRN2 ISA allows only one sync wait per instruction.
    nc = bacc.Bacc(None, target_bir_lowering=False, debug=False)
    yp_d = nc.declare_dram_parameter("yp", [SLAB, H, W], bf16, isOutput=False)
    yt_d = nc.declare_dram_parameter("yt", [HSLAB, H, W], bf16, isOutput=False)
    xx_d = nc.declare_dram_parameter("xx", [HSLAB, H, W], bf16, isOutput=False)
    bd_d = nc.declare_dram_parameter("band", [128, 512], fp32, isOutput=False)
    out_d = nc.declare_dram_parameter("out", [128, SLAB // 2], fp32, isOutput=True)

    NB = 4  # slices batched per instruction (quad)

    def dram_quad(t, i):
        # slices [i, i+NB) -> [128 part, NB (d), 2 (j), 256 (w)], partition p
        # holds H rows (2p, 2p+1): per-partition run = 512 bf16 = 1 KiB
        return t[i:i + NB].rearrange("d (p j) w -> p d j w", p=128)

    with TileContext(nc) as tc:
        with (
            tc.tile_pool(name="pconst", bufs=1) as pconst,
            tc.tile_pool(name="pio", bufs=1) as pio,
            tc.tile_pool(name="pwork", bufs=1) as pwork,
            tc.tile_pool(name="pps", bufs=2, space="PSUM") as pps,
        ):
            band_f = pconst.tile([128, 512], fp32, tag="band_f")
            nc.sync.dma_start(out=band_f[:, :], in_=bd_d[:, :])
            band_b = pconst.tile([128, 512], bf16, tag="band_b")
            nc.vector.tensor_copy(out=band_b[:, :], in_=band_f[:, :])

            def bblk(b_k, b_m):
                o = 128 * (2 * b_k + b_m)
                return band_b[:, o:o + 128]

            # per-quad partial sums; cols 2k = sum(sq), 2k+1 = sum(sq*bh)
            acc = pconst.tile([128, SLAB // 2], fp32, tag="accA")

            y1_t = {}    # quad-base j -> tile [128,NB,2,256] for (j..j+3)
            h0_t = {}
            yt_t = {}
            bone_t = {}  # quad-base d -> tile for (d..d+3)
            T_t = {}

            def S(quads, i):
                # slice view of a quad tile, [128, 2, 256]
                return quads[i - i % NB][:, i % NB, :, :]

            def load_quad(j):
                yt = pio.tile([128, NB, 2, W], bf16, tag="yt", bufs=4)
                nc.sync.dma_start(out=yt[:, :, :, :], in_=dram_quad(yt_d, j))
                xv = pio.tile([128, NB, 2, W], bf16, tag="xv", bufs=2)
                nc.sync.dma_start(out=xv[:, :, :, :], in_=dram_quad(xx_d, j))
                # ts gets the DVE 4x mode, tt gets 2x; scalar_tensor_tensor
                # only has a 1x uop -- avoid it on the bottleneck engine.
                y1 = pwork.tile([128, NB, 2, W], bf16, tag="y1", bufs=2)
                nc.vector.tensor_scalar(y1[:, :, :, :], yt[:, :, :, :], 0.5, None, OP.is_ge)
                x1 = pwork.tile([128, NB, 2, W], bf16, tag="x1", bufs=2)
                nc.vector.tensor_scalar(x1[:, :, :, :], xv[:, :, :, :], 0.5, None, OP.is_ge)
                # hole0 = y1 & ~x1
                h0 = pwork.tile([128, NB, 2, W], bf16, tag="h0", bufs=6)
                nc.vector.tensor_tensor(h0[:, :, :, :], y1[:, :, :, :], x1[:, :, :, :], OP.is_gt)
                yt_t[j], y1_t[j], h0_t[j] = yt, y1, h0
                # bone[d] = x1[d+8] | y1[d+8]; quad d0 = j - 8
                d0 = j - LPAD
                if 0 <= d0 < SLAB:
                    bone = pwork.tile([128, NB, 2, W], bf16, tag="bone", bufs=3)
                    nc.vector.tensor_tensor(bone[:, :, :, :], y1[:, :, :, :], x1[:, :, :, :], OP.max)
                    bone_t[d0] = bone

            def d_sum(d):
                # T[d] = sum_{j in d+1 .. d+15} h0[j]  (bf16 ints <= 15, exact)
                if d % NB == 0:
                    T_t[d] = pwork.tile([128, NB, 2, W], bf16, tag="T", bufs=2,
                                        name=f"T{d}")
                T = S(T_t, d)
                if d == 0:
                    nc.vector.tensor_tensor(T, S(h0_t, 1), S(h0_t, 2), OP.add)
                    for j in range(3, 16):
                        nc.vector.tensor_tensor(T, T, S(h0_t, j), OP.add)
                else:
                    nc.vector.tensor_tensor(T, S(T_t, d - 1), S(h0_t, d + 15), OP.add)
                    nc.vector.tensor_tensor(T, T, S(h0_t, d), OP.subtract)

            def hole_quad(d):
                # d % 4 == 0: H-matmul + threshold + W max tree, slices d..d+3
                Tp = T_t[d]
                # psum quad [128, 2(b_m), NB(dd), 256]: each MM writes N=512
                # into one PSUM bank (b_m, dd-half)
                ps = pps.tile([128, 2, NB, W], fp32, tag="ps")
                for b_m in (0, 1):
                    for h in (0, 1):
                        for b_k in (0, 1):
                            nc.tensor.matmul(
                                ps[:, b_m, 2 * h:2 * h + 2, :],
                                bblk(b_k, b_m),
                                Tp[:, 2 * h:2 * h + 2, b_k, :],
                                start=(b_k == 0), stop=(b_k == 1))
                # threshold: counts > 0 -> 1; write into padded tiles
                hh = pwork.tile([128, NB, 2, WPAD], bf16, tag="hh", bufs=2)
                nc.gpsimd.memset(hh[:, :, :, 0:HALO], 0.0)
                nc.gpsimd.memset(hh[:, :, :, W + HALO:WPAD], 0.0)
                # ps is [p, b_m(j), dd, w]; hh is [p, dd, j, w]
                nc.scalar.activation(
                    hh[:, :, :, HALO:W + HALO],
                    ps[:, :, :, :].rearrange("p b d w -> p d b w"),
                    ACT.Sign,
                )
                w1 = pwork.tile([128, NB, 2, WPAD], bf16, tag="w1", bufs=2)
                nc.vector.tensor_tensor(w1[:, :, :, 0:269], hh[:, :, :, 0:269], hh[:, :, :, 1:270], OP.max)
                w2 = pwork.tile([128, NB, 2, WPAD], bf16, tag="w2", bufs=2)
                nc.vector.tensor_tensor(w2[:, :, :, 0:267], w1[:, :, :, 0:267], w1[:, :, :, 2:269], OP.max)
                w3 = pwork.tile([128, NB, 2, WPAD], bf16, tag="w3", bufs=2)
                nc.vector.tensor_tensor(w3[:, :, :, 0:263], w2[:, :, :, 0:263], w2[:, :, :, 4:267], OP.max)
                hd = pwork.tile([128, NB, 2, W], bf16, tag="hd", bufs=2)
                nc.vector.tensor_tensor(hd[:, :, :, :], w3[:, :, :, 0:W], w3[:, :, :, HALO:W + HALO], OP.max)
                return hd

            def combine_quad(d, hd):
                # diff/sq/weights for slices d..d+3; yt quad j = d+8
                yp = pio.tile([128, NB, 2, W], bf16, tag="yp", bufs=2)
                nc.sync.dma_start(out=yp[:, :, :, :], in_=dram_quad(yp_d, d))
                diff = pwork.tile([128, NB, 2, W], bf16, tag="diff", bufs=2)
                nc.vector.tensor_tensor(diff[:, :, :, :], yp[:, :, :, :], yt_t[d + LPAD][:, :, :, :], OP.subtract)
                # sq = diff^2 with accumulated sum(sq) on the Scalar engine
                q = d // NB
                sq = pwork.tile([128, NB, 2, W], bf16, tag="sq", bufs=2)
                nc.scalar.activation(sq[:, :, :, :], diff[:, :, :, :], ACT.Square,
                                     accum_out=acc[:, 2 * q:2 * q + 1])
                # weight w = 1 + bone + hole_dil: sum(sq*w) = sum(sq) + sum(sq*bh)
                bh = pwork.tile([128, NB, 2, W], bf16, tag="bh", bufs=2)
                nc.vector.tensor_tensor(bh[:, :, :, :], bone_t[d][:, :, :, :], hd[:, :, :, :], OP.add)
                prod = pwork.tile([128, NB, 2, W], bf16, tag="prod", bufs=2)
                nc.vector.tensor_tensor(prod[:, :, :, :], sq[:, :, :, :], bh[:, :, :, :], OP.mult)
                scr = pwork.tile([128, NB, 2, W], bf16, tag="scr", bufs=2)
                nc.scalar.activation(scr[:, :, :, :], prod[:, :, :, :], ACT.Copy,
                                     accum_out=acc[:, 2 * q + 1:2 * q + 2])

            next_d = 0
            next_hole = 0
            for jq in range(0, HSLAB, NB):
                load_quad(jq)
                while next_d < SLAB and next_d + 15 <= jq + NB - 1:
                    d_sum(next_d)
                    next_d += 1
                while next_hole + NB - 1 < next_d:
                    hd = hole_quad(next_hole)
                    combine_quad(next_hole, hd)
                    next_hole += NB

            nc.sync.dma_start(out=out_d[:, :], in_=acc[:, :])

    nc.finalize()
    return nc


def _get_nc():
    if "nc" not in _NC_CACHE:
        _NC_CACHE["nc"] = _build_nc()
    return _NC_CACHE["nc"]


def _install_profile_bridge():
    """Register the axon NTFF profile hook that the image's antenv lacks,
    and stub out the S3 artifact upload (no creds in this container)."""
    import types

    import concourse.bass_utils as bu

    if "antenv.axon_hooks" not in sys.modules:
        try:
            from trn_agent_boot.trn_boot import _ntff_profile_via_ctypes

            hook = _ntff_profile_via_ctypes("/opt/axon/libaxon_pjrt.so")
            mod = types.ModuleType("antenv.axon_hooks")
            mod.get_axon_ntff_profile_hook = lambda: hook
            mod.set_axon_ntff_profile_hook = lambda h: None
            sys.modules["antenv.axon_hooks"] = mod
            import antenv

            antenv.axon_hooks = mod
        except Exception as e:  # degrade to trace-less run
            print(f"profile bridge unavailable: {e}", file=sys.stderr)
    bu.upload_artifacts = lambda tmpdir: tmpdir


def kernel(y_pred, y_true, x):
    global LAST_EXEC_NS, LAST_RESULT
    import ml_dtypes

    bf = ml_dtypes.bfloat16
    yp = np.asarray(y_pred, dtype=np.float32).reshape(D_FULL, H, W).astype(bf)
    yt = np.asarray(y_true, dtype=np.float32).reshape(D_FULL, H, W).astype(bf)
    xv = np.asarray(x, dtype=np.float32).reshape(D_FULL, H, W).astype(bf)

    band = _band_blocks()
    in_maps = []
    for c in range(NCORES):
        g0 = c * SLAB - LPAD
        yt_s = np.zeros((HSLAB, H, W), bf)
        xx_s = np.zeros((HSLAB, H, W), bf)
        lo, hi = max(0, g0), min(D_FULL, g0 + HSLAB)
        yt_s[lo - g0:hi - g0] = yt[lo:hi]
        xx_s[lo - g0:hi - g0] = xv[lo:hi]
        in_maps.append({
            "yp": np.ascontiguousarray(yp[c * SLAB:(c + 1) * SLAB]),
            "yt": yt_s,
            "xx": xx_s,
            "band": band,
        })

    from concourse.bass_utils import run_bass_kernel_spmd

    nc = _get_nc()
    trace = os.environ.get("KERNEL_TRACE", "0") == "1"
    if trace:
        _install_profile_bridge()
    res = run_bass_kernel_spmd(nc, in_maps, list(range(NCORES)), trace=trace)
    LAST_EXEC_NS = res.exec_time_ns
    LAST_RESULT = res

    tot = 0.0
    for r in res.results:
        o = np.asarray(r["out"], dtype=np.float64)
        tot += o.sum()
    return np.asarray(tot / NTOT, dtype=np.float32)

